# revision 1
# baseline (speedup 1.0000x reference)
"""Trainium2 Bass kernel for nn_DNA_32916629356554 (moe_routing).

Reference model (T=2048 tokens, D=1024, E=8 experts, H=4096, 4 hops, top-2):
  h = embed_W[ids]
  4x hop: logits = rmsnorm(h, rln) @ rW ; probs = top2-masked softmax
          per-expert: u_e = gelu(rmsnorm(h, eln_e) @ W1_e); y_e = u_e @ W2_e
          h += sum_e probs[:,e] * y_e
  out = rmsnorm(h, oln) @ embed_W.T

Sharding: expert-parallel over 8 cores (1 expert per core, AllReduce the
weighted delta per token-block), vocab-parallel final matmul (4000 cols/core).

Layout: h kept feature-major h_T [D, T] fp32 in SBUF.  rmsnorm scale via
ones-matmul partition reduction (fp32r), broadcast via ones-matmul (fp32).
Router logits via true-fp32 matmul with ln weights folded into rW on host and
the per-token rmsnorm scale applied to the logits afterwards (exact rewrite).
Expert MLP and final tied-embedding matmul run in fp32r (full PE rate at
N>=256, ~1e-4 relative error) to keep precision-induced routing flips rare.
"""

import os
import numpy as np

DBG_HOPS = int(os.environ.get("KDBG_HOPS", "4"))
DBG_FINAL = int(os.environ.get("KDBG_FINAL", "1"))
DBG_GATHER = int(os.environ.get("KDBG_GATHER", "1"))
DBG_AR = int(os.environ.get("KDBG_AR", "1"))
DBG_STAGE = int(os.environ.get("KDBG_STAGE", "99"))

# split-bf16 matmul passes for the expert MLP (hi/lo operand splits).
# W1 passes: subsets of [hh, hl, lh] meaning (W1 level, x level).
W1_XLO = int(os.environ.get("KP_W1_XLO", "1"))   # include W1h * x_lo
W1_WLO = int(os.environ.get("KP_W1_WLO", "1"))   # include W1l * x_hi
W2_ULO = int(os.environ.get("KP_W2_ULO", "1"))   # include W2h * u_lo
W2_WLO = int(os.environ.get("KP_W2_WLO", "1"))   # include W2l * u_hi

V, D, E, H, T = 32000, 1024, 8, 4096, 2048
NHOPS = 4
NCORES = 8
VS = V // NCORES      # vocab shard per core
TB = 1024             # token block (collective granularity)
NB = T // TB
SB = 512              # MLP sub-block (psum N)
EPS = 1e-5

_CACHE = {}


def _build():
    import concourse.bass as bass
    import concourse.mybir as mybir
    import concourse.tile as tile
    from concourse import bacc
    from concourse.masks import make_identity

    dt = mybir.dt
    f32, f32r, i32 = dt.float32, dt.float32r, dt.int32
    bf16 = dt.bfloat16
    if os.environ.get("KDBG_PREC", "") == "f32":
        f32r = dt.float32
    AF = mybir.ActivationFunctionType
    Alu = mybir.AluOpType

    nc = bacc.Bacc(num_devices=NCORES)

    ids = nc.declare_dram_parameter("ids", [T], i32, isOutput=False)
    embed = nc.declare_dram_parameter("embed", [V, D], f32, isOutput=False)
    embT = nc.declare_dram_parameter("embT", [D, VS], bf16, isOutput=False)
    rw = nc.declare_dram_parameter("rw", [NHOPS, D, E], f32, isOutput=False)
    w1h = nc.declare_dram_parameter("w1h", [D, H], bf16, isOutput=False)
    w1l = nc.declare_dram_parameter("w1l", [D, H], bf16, isOutput=False)
    w2h = nc.declare_dram_parameter("w2h", [H, D], bf16, isOutput=False)
    w2l = nc.declare_dram_parameter("w2l", [H, D], bf16, isOutput=False)
    sel = nc.declare_dram_parameter("sel", [E], f32, isOutput=False)
    out = nc.declare_dram_parameter("out", [T, VS], f32, isOutput=True)

    KD = D // 128   # 8 d-chunks
    KH = H // 128   # 32 h-chunks
    NT = T // 128   # 16 token tiles
    NV = 8          # vocab chunks
    VC = VS // NV   # 500

    with tile.TileContext(nc) as tc:
        with tc.tile_pool(name="persist", bufs=1) as per:
            identity = per.tile([128, 128], f32)
            make_identity(nc, identity)

            ids_sb = per.tile([128, NT], i32)
            nc.sync.dma_start(out=ids_sb, in_=ids[:].rearrange("(n p) -> p n", p=128))

            sel_sb = per.tile([128, E], f32)
            sel_ap = sel[:]
            nc.sync.dma_start(
                out=sel_sb,
                in_=bass.AP(tensor=sel_ap.tensor, offset=0, ap=[[0, 128], [1, E]]),
            )

            eps1 = per.tile([1, 1], f32)
            nc.vector.memset(eps1, EPS)

            ones_k_f = per.tile([128, 1], f32)
            nc.vector.memset(ones_k_f, 1.0)
            ones_k = per.tile([128, 1], f32r)
            nc.vector.tensor_copy(out=ones_k[:], in_=ones_k_f[:])
            ones_1 = per.tile([1, 128], f32)
            nc.vector.memset(ones_1, 1.0)

            rw_sb = []
            for hop in range(NHOPS):
                t_ = per.tile([128, KD, E], f32, name=f"rw_sb{hop}")
                nc.sync.dma_start(
                    out=t_, in_=rw[hop].rearrange("(kd p) e -> p kd e", p=128)
                )
                rw_sb.append(t_)

            # persistent feature-major hidden state: h_T[d] = [128, T]
            h_T = [per.tile([128, T], f32, name=f"h_T{d}") for d in range(KD)]

            # ---- embedding gather + transpose to feature-major ----
            with (
                tc.tile_pool(name="init", bufs=2) as init,
                tc.tile_pool(name="init_ps", bufs=2, space="PSUM") as init_ps,
            ):
                # warm PE's vector clock on identity (avoids multi-wait matmuls)
                warm_ps = init_ps.tile([128, 128], f32, tag="tr", bufs=3)
                nc.tensor.transpose(
                    out=warm_ps[:], in_=identity[:], identity=identity[:]
                )
                for p in range(NT):
                    h0 = init.tile([128, D], f32, tag="h0", bufs=3)
                    if DBG_GATHER:
                        nc.gpsimd.indirect_dma_start(
                            out=h0[:],
                            out_offset=None,
                            in_=embed[:],
                            in_offset=bass.IndirectOffsetOnAxis(
                                ap=ids_sb[:, p : p + 1], axis=0
                            ),
                        )
                    else:
                        nc.sync.dma_start(
                            out=h0, in_=embed[p * 128 : (p + 1) * 128, :]
                        )
                    for d in range(KD):
                        tr_ps = init_ps.tile([128, 128], f32, tag="tr", bufs=3)
                        nc.tensor.transpose(
                            out=tr_ps[:],
                            in_=h0[:, d * 128 : (d + 1) * 128],
                            identity=identity[:],
                        )
                        nc.vector.tensor_copy(
                            out=h_T[d][:, p * 128 : (p + 1) * 128], in_=tr_ps[:]
                        )

            # ---- helper: rmsnorm scale for a token block -> s_bc [128, TB] f32
            def scale_block(work, psum, b):
                t0 = b * TB
                srow = work.tile([1, TB], f32, tag="srow", bufs=1)
                for ch in range(TB // 512):
                    c0 = t0 + ch * 512
                    ss_ps = psum.tile([1, 512], f32, tag="ssps", bufs=1)
                    for d in range(KD):
                        hsq = work.tile([128, 512], f32r, tag="hsq", bufs=1)
                        nc.vector.tensor_mul(
                            hsq[:], h_T[d][:, c0 : c0 + 512], h_T[d][:, c0 : c0 + 512]
                        )
                        nc.tensor.matmul(
                            out=ss_ps[:],
                            lhsT=ones_k[:],
                            rhs=hsq[:],
                            start=(d == 0),
                            stop=(d == KD - 1),
                        )
                    # sqrt(ss/D + eps)
                    nc.scalar.activation(
                        out=srow[:, ch * 512 : (ch + 1) * 512],
                        in_=ss_ps[:],
                        func=AF.Sqrt,
                        bias=eps1[:],
                        scale=1.0 / D,
                    )
                srec = srow
                nc.vector.reciprocal(out=srec[:], in_=srow[:])
                s_bc = work.tile([128, TB], f32, tag="sbc", bufs=1)
                for ch in range(TB // 512):
                    sp = psum.tile([128, 512], f32, tag="smallps", bufs=1)
                    nc.tensor.matmul(
                        out=sp[:],
                        lhsT=ones_1[:],
                        rhs=srec[:, ch * 512 : (ch + 1) * 512],
                        start=True,
                        stop=True,
                    )
                    nc.vector.tensor_copy(
                        out=s_bc[:, ch * 512 : (ch + 1) * 512], in_=sp[:]
                    )
                return s_bc

            # ---- hops ----
            with (
                tc.tile_pool(name="work", bufs=2) as work,
                tc.tile_pool(name="hops_ps", bufs=2, space="PSUM") as psum,
                tc.tile_pool(name="dram", bufs=2, space="DRAM") as dramp,
            ):
                for hop in range(DBG_HOPS):
                    for b in range(NB):
                        t0 = b * TB
                        tsl = slice(t0, t0 + TB)
                        s_bc = scale_block(work, psum, b)
                        if DBG_STAGE < 3:
                            continue
                        # router logits (fp32), scale applied post-matmul
                        lg_sb = work.tile([E, TB], f32, tag="lg", bufs=1)
                        for ch in range(TB // 512):
                            c0 = t0 + ch * 512
                            lg_ps = psum.tile([E, 512], f32, tag="smallps", bufs=1)
                            for d in range(KD):
                                nc.tensor.matmul(
                                    out=lg_ps[:],
                                    lhsT=rw_sb[hop][:, d, :],
                                    rhs=h_T[d][:, c0 : c0 + 512],
                                    start=(d == 0),
                                    stop=(d == KD - 1),
                                )
                            nc.vector.tensor_mul(
                                lg_sb[:, ch * 512 : (ch + 1) * 512],
                                lg_ps[:],
                                s_bc[:E, ch * 512 : (ch + 1) * 512],
                            )

                        if DBG_STAGE < 4:
                            continue
                        # top-2 masked softmax -> this core's prob column
                        prow = work.tile([1, TB], f32, tag="prow", bufs=1)
                        for p in range(TB // 128):
                            lgt_ps = psum.tile([128, E], f32, tag="smallps", bufs=1)
                            nc.tensor.transpose(
                                out=lgt_ps[:],
                                in_=lg_sb[:, p * 128 : (p + 1) * 128],
                                identity=identity[:E, :E],
                            )
                            lgt = work.tile([128, E], f32, tag="lgt", bufs=3)
                            nc.vector.tensor_copy(out=lgt[:], in_=lgt_ps[:])
                            m1n = work.tile([128, 1], f32, tag="m1n", bufs=3)
                            nc.vector.tensor_reduce(
                                out=m1n[:],
                                in_=lgt[:],
                                axis=mybir.AxisListType.X,
                                op=Alu.max,
                                negate=True,
                            )
                            d1 = work.tile([128, E], f32, tag="d1", bufs=3)
                            nc.vector.tensor_scalar_add(d1[:], lgt[:], m1n[:])
                            mask1 = work.tile([128, E], f32, tag="mask1", bufs=3)
                            nc.vector.tensor_scalar(
                                mask1[:], d1[:], 0.0, None, op0=Alu.is_equal
                            )
                            masked = work.tile([128, E], f32, tag="masked", bufs=3)
                            nc.vector.scalar_tensor_tensor(
                                out=masked[:],
                                in0=mask1[:],
                                scalar=-1e30,
                                in1=lgt[:],
                                op0=Alu.mult,
                                op1=Alu.add,
                            )
                            m2 = work.tile([128, 1], f32, tag="m2", bufs=3)
                            nc.vector.tensor_reduce(
                                out=m2[:],
                                in_=masked[:],
                                axis=mybir.AxisListType.X,
                                op=Alu.max,
                            )
                            top2 = work.tile([128, E], f32, tag="top2", bufs=3)
                            nc.vector.tensor_scalar(
                                top2[:], lgt[:], m2[:], None, op0=Alu.is_ge
                            )
                            e_t = work.tile([128, E], f32, tag="e_t", bufs=3)
                            se = work.tile([128, 1], f32, tag="se", bufs=3)
                            nc.scalar.activation(
                                out=e_t[:], in_=d1[:], func=AF.Exp, accum_out=se[:]
                            )
                            rse = work.tile([128, 1], f32, tag="rse", bufs=3)
                            nc.vector.reciprocal(out=rse[:], in_=se[:])
                            probs = work.tile([128, E], f32, tag="probs", bufs=3)
                            nc.vector.scalar_tensor_tensor(
                                out=probs[:],
                                in0=e_t[:],
                                scalar=rse[:],
                                in1=top2[:],
                                op0=Alu.mult,
                                op1=Alu.mult,
                            )
                            pscr = work.tile([128, E], f32, tag="pscr", bufs=3)
                            pcol = work.tile([128, 1], f32, tag="pcol", bufs=3)
                            nc.vector.tensor_mul(pscr[:], probs[:], sel_sb[:])
                            nc.vector.reduce_sum(
                                out=pcol[:], in_=pscr[:], axis=mybir.AxisListType.X
                            )
                            nc.sync.dma_start(
                                out=prow[0:1, p * 128 : (p + 1) * 128], in_=pcol[:]
                            )

                        if DBG_STAGE < 5:
                            continue
                        pb = work.tile([128, TB], f32, tag="pb", bufs=1)
                        for ch in range(TB // 512):
                            pp = psum.tile([128, 512], f32, tag="smallps", bufs=1)
                            nc.tensor.matmul(
                                out=pp[:],
                                lhsT=ones_1[:],
                                rhs=prow[:, ch * 512 : (ch + 1) * 512],
                                start=True,
                                stop=True,
                            )
                            nc.vector.tensor_copy(
                                out=pb[:, ch * 512 : (ch + 1) * 512], in_=pp[:]
                            )

                        if DBG_STAGE < 6:
                            continue
                        # expert MLP + weighted delta (split-bf16 multi-pass)
                        din = dramp.tile([D, TB], f32, tag="din", bufs=2)
                        dout = dramp.tile(
                            [D, TB], f32, tag="dout", bufs=2, addr_space="Shared"
                        )
                        w1h_r = w1h[:].rearrange("(kd p) h -> p kd h", p=128)
                        w1l_r = w1l[:].rearrange("(kd p) h -> p kd h", p=128)
                        w2h_r = w2h[:].rearrange("(kh p) dd -> p kh dd", p=128)
                        w2l_r = w2l[:].rearrange("(kh p) dd -> p kh dd", p=128)
                        nsub = TB // SB
                        for sub in range(nsub):
                            ssl = slice(t0 + sub * SB, t0 + (sub + 1) * SB)
                            bsl = slice(sub * SB, (sub + 1) * SB)
                            xh, xl = [], []
                            for d in range(KD):
                                xnf = work.tile([128, SB], f32, tag="xnf", bufs=1)
                                nc.vector.tensor_mul(
                                    xnf[:], h_T[d][:, ssl], s_bc[:, bsl]
                                )
                                xh_ = work.tile(
                                    [128, SB], bf16, tag=f"xh{d}", bufs=1,
                                    name=f"xh_{d}",
                                )
                                nc.vector.tensor_copy(out=xh_[:], in_=xnf[:])
                                xh.append(xh_)
                                if W1_XLO:
                                    xhf = work.tile(
                                        [128, SB], f32, tag="xhf", bufs=1
                                    )
                                    nc.vector.tensor_copy(out=xhf[:], in_=xh_[:])
                                    xl_ = work.tile(
                                        [128, SB], bf16, tag=f"xl{d}", bufs=1,
                                        name=f"xl_{d}",
                                    )
                                    nc.vector.tensor_sub(xl_[:], xnf[:], xhf[:])
                                    xl.append(xl_)
                            uh, ul = [], []
                            for m in range(KH):
                                w1ht = work.tile(
                                    [128, KD, 128], bf16, tag="w1ht", bufs=2
                                )
                                nc.sync.dma_start(
                                    out=w1ht,
                                    in_=w1h_r[:, :, m * 128 : (m + 1) * 128],
                                )
                                if W1_WLO:
                                    w1lt = work.tile(
                                        [128, KD, 128], bf16, tag="w1lt", bufs=2
                                    )
                                    nc.sync.dma_start(
                                        out=w1lt,
                                        in_=w1l_r[:, :, m * 128 : (m + 1) * 128],
                                    )
                                u_ps = psum.tile([128, SB], f32, tag="ups", bufs=3)
                                steps = []
                                for kd in range(KD):
                                    steps.append((w1ht[:, kd, :], xh[kd][:]))
                                    if W1_XLO:
                                        steps.append((w1ht[:, kd, :], xl[kd][:]))
                                    if W1_WLO:
                                        steps.append((w1lt[:, kd, :], xh[kd][:]))
                                ns = len(steps)
                                for i, (lw, rx) in enumerate(steps):
                                    nc.tensor.matmul(
                                        out=u_ps[:], lhsT=lw, rhs=rx,
                                        start=(i == 0), stop=(i == ns - 1),
                                    )
                                uf = work.tile([128, SB], f32, tag="scr", bufs=2)
                                nc.scalar.activation(
                                    out=uf[:], in_=u_ps[:], func=AF.Gelu_apprx_tanh
                                )
                                uh_ = work.tile(
                                    [128, SB], bf16, tag=f"uh{m}", bufs=1,
                                    name=f"uh_{m}",
                                )
                                nc.vector.tensor_copy(out=uh_[:], in_=uf[:])
                                uh.append(uh_)
                                if W2_ULO:
                                    uhf = work.tile(
                                        [128, SB], f32, tag="uhf", bufs=1
                                    )
                                    nc.vector.tensor_copy(out=uhf[:], in_=uh_[:])
                                    ul_ = work.tile(
                                        [128, SB], bf16, tag=f"ul{m}", bufs=1,
                                        name=f"ul_{m}",
                                    )
                                    nc.vector.tensor_sub(ul_[:], uf[:], uhf[:])
                                    ul.append(ul_)
                            for d in range(KD):
                                y_ps = psum.tile([128, SB], f32, tag="yps", bufs=3)
                                npass = 1 + W2_ULO + W2_WLO
                                total = KH * npass
                                step = 0
                                for q in range(4):
                                    w2ht = work.tile(
                                        [128, KH // 4, 128], bf16, tag="w2ht",
                                        bufs=2,
                                    )
                                    nc.sync.dma_start(
                                        out=w2ht,
                                        in_=w2h_r[
                                            :,
                                            q * (KH // 4) : (q + 1) * (KH // 4),
                                            d * 128 : (d + 1) * 128,
                                        ],
                                    )
                                    if W2_WLO:
                                        w2lt = work.tile(
                                            [128, KH // 4, 128], bf16, tag="w2lt",
                                            bufs=2,
                                        )
                                        nc.sync.dma_start(
                                            out=w2lt,
                                            in_=w2l_r[
                                                :,
                                                q * (KH // 4) : (q + 1) * (KH // 4),
                                                d * 128 : (d + 1) * 128,
                                            ],
                                        )
                                    for kh in range(KH // 4):
                                        k = q * (KH // 4) + kh
                                        mmset = [(w2ht[:, kh, :], uh[k][:])]
                                        if W2_ULO:
                                            mmset.append(
                                                (w2ht[:, kh, :], ul[k][:])
                                            )
                                        if W2_WLO:
                                            mmset.append(
                                                (w2lt[:, kh, :], uh[k][:])
                                            )
                                        for lw, rx in mmset:
                                            nc.tensor.matmul(
                                                out=y_ps[:], lhsT=lw, rhs=rx,
                                                start=(step == 0),
                                                stop=(step == total - 1),
                                            )
                                            step += 1
                                delta = work.tile(
                                    [128, SB], f32, tag="delta", bufs=1
                                )
                                nc.vector.tensor_mul(delta[:], y_ps[:], pb[:, bsl])
                                nc.sync.dma_start(
                                    out=din[d * 128 : (d + 1) * 128, bsl],
                                    in_=delta[:],
                                )

                        if DBG_AR:
                            nc.gpsimd.collective_compute(
                                "AllReduce",
                                Alu.add,
                                replica_groups=[list(range(NCORES))],
                                ins=[din.opt()],
                                outs=[dout.opt()],
                            )
                        src_buf = dout if DBG_AR else din
                        for d in range(KD):
                            ar_t = work.tile([128, TB], f32, tag="ar", bufs=1)
                            nc.sync.dma_start(
                                out=ar_t, in_=src_buf[d * 128 : (d + 1) * 128, :]
                            )
                            nc.vector.tensor_add(
                                h_T[d][:, tsl], h_T[d][:, tsl], ar_t[:]
                            )

            # ---- final: out = rmsnorm(h) @ embT (out-ln folded into embT) ----
            if not DBG_FINAL:
                with tc.tile_pool(name="fin0", bufs=2) as fin0:
                    for t in range(NT):
                        dbg = fin0.tile([128, VS], f32, tag="dbg", bufs=2)
                        nc.vector.memset(dbg, 0.0)
                        for d in range(KD):
                            nc.vector.tensor_copy(
                                out=dbg[:, d * 128 : (d + 1) * 128],
                                in_=h_T[d][:, t * 128 : (t + 1) * 128],
                            )
                        nc.sync.dma_start(
                            out=out[t * 128 : (t + 1) * 128, :], in_=dbg[:]
                        )
            else:
                with (
                    tc.tile_pool(name="fin", bufs=2) as fin,
                    tc.tile_pool(name="fin_ps", bufs=2, space="PSUM") as fpsum,
                ):
                    xnf = [
                        fin.tile(
                            [128, T], bf16, tag=f"xnf{d}", bufs=1, name=f"xnf_{d}"
                        )
                        for d in range(KD)
                    ]
                    for b in range(NB):
                        tsl = slice(b * TB, (b + 1) * TB)
                        s_bc = scale_block(fin, fpsum, b)
                        for d in range(KD):
                            nc.vector.tensor_mul(
                                xnf[d][:, tsl], h_T[d][:, tsl], s_bc[:]
                            )

                    embT_r = embT[:].rearrange("(kd p) v -> p kd v", p=128)
                    for v in range(NV):
                        et = fin.tile([128, KD, VC], bf16, tag="et", bufs=3)
                        nc.sync.dma_start(
                            out=et, in_=embT_r[:, :, v * VC : (v + 1) * VC]
                        )
                        for t in range(NT):
                            o_ps = fpsum.tile([128, VC], f32, tag="ops", bufs=4)
                            for kd in range(KD):
                                nc.tensor.matmul(
                                    out=o_ps[:],
                                    lhsT=xnf[kd][:, t * 128 : (t + 1) * 128],
                                    rhs=et[:, kd, :],
                                    start=(kd == 0),
                                    stop=(kd == KD - 1),
                                )
                            o_sb = fin.tile([128, VC], f32, tag="osb", bufs=4)
                            nc.scalar.copy(out=o_sb[:], in_=o_ps[:])
                            nc.sync.dma_start(
                                out=out[
                                    t * 128 : (t + 1) * 128, v * VC : (v + 1) * VC
                                ],
                                in_=o_sb[:],
                            )

    nc.finalize()
    return nc


def _split_bf16(a):
    import ml_dtypes

    ah = a.astype(ml_dtypes.bfloat16)
    al = (a - ah.astype(np.float32)).astype(ml_dtypes.bfloat16)
    return np.ascontiguousarray(ah), np.ascontiguousarray(al)


def _prep_inputs(ids, embed_W, router_ln_w, router_W, expert_ln_w, expert_W1,
                 expert_W2, ln_out_w):
    """Host-side sharding + layout/constant-folding prep. Returns in_maps."""
    import ml_dtypes

    ids = np.ascontiguousarray(np.asarray(ids, dtype=np.int32))
    embed_W = np.ascontiguousarray(np.asarray(embed_W, dtype=np.float32))
    # fold router ln weight into router weights (rmsnorm scale applied on-device)
    rwf = np.ascontiguousarray(
        np.asarray(router_W, np.float32)
        * np.asarray(router_ln_w, np.float32)[:, :, None]
    )
    # fold out-ln into transposed embedding
    embT_full = np.ascontiguousarray(
        (embed_W * np.asarray(ln_out_w, np.float32)[None, :]).T
    )

    in_maps = []
    for c in range(NCORES):
        w1f = np.ascontiguousarray(
            np.asarray(expert_W1[c], np.float32)
            * np.asarray(expert_ln_w[c], np.float32)[:, None]
        )
        w2f = np.ascontiguousarray(np.asarray(expert_W2[c], np.float32))
        w1hc, w1lc = _split_bf16(w1f)
        w2hc, w2lc = _split_bf16(w2f)
        embTc = np.ascontiguousarray(
            embT_full[:, c * VS : (c + 1) * VS].astype(ml_dtypes.bfloat16)
        )
        selc = np.zeros((E,), np.float32)
        selc[c] = 1.0
        in_maps.append(
            {
                "ids": ids,
                "embed": embed_W,
                "embT": embTc,
                "rw": rwf,
                "w1h": w1hc,
                "w1l": w1lc,
                "w2h": w2hc,
                "w2l": w2lc,
                "sel": selc,
            }
        )
    return in_maps


def _get_nc():
    if "nc" not in _CACHE:
        _CACHE["nc"] = _build()
    return _CACHE["nc"]


def run(inputs, trace=False):
    """Run on 8 NeuronCores; returns (full_output, BassKernelResults)."""
    from concourse.bass_utils import run_bass_kernel_spmd

    k = int(inputs.get("k", 2))
    assert k == 2, f"kernel hardcodes top-2 routing, got k={k}"
    in_maps = _prep_inputs(
        inputs["ids"], inputs["embed_W"], inputs["router_ln_w"], inputs["router_W"],
        inputs["expert_ln_w"], inputs["expert_W1"], inputs["expert_W2"],
        inputs["ln_out_w"],
    )
    nc = _get_nc()
    res = run_bass_kernel_spmd(nc, in_maps, list(range(NCORES)), trace=trace)
    full = np.concatenate([res.results[c]["out"] for c in range(NCORES)], axis=1)
    return full, res


def kernel(**inputs) -> np.ndarray:
    full, _ = run(inputs, trace=False)
    return full



# revision 7
# speedup vs baseline: 3.6215x; 3.6215x over previous
"""Trainium2 Bass kernel for nn_DNA_32916629356554 (moe_routing).

Reference model (T=2048 tokens, D=1024, E=8 experts, H=4096, 4 hops, top-2):
  h = embed_W[ids]
  4x hop: logits = rmsnorm(h, rln) @ rW ; probs = top2-masked softmax
          per-expert: u_e = gelu(rmsnorm(h, eln_e) @ W1_e); y_e = u_e @ W2_e
          h += sum_e probs[:,e] * y_e
  out = rmsnorm(h, oln) @ embed_W.T

Sharding: expert-parallel over 8 cores (1 expert per core, AllReduce the
weighted delta per token-block), vocab-parallel final matmul (4000 cols/core).

Layout: h kept feature-major h_T [D, T] fp32 in SBUF.  rmsnorm scale via
ones-matmul partition reduction, broadcast via ones-matmul.  Router logits via
true-fp32 matmul with ln weights folded into rW on host and the per-token
rmsnorm scale applied to the logits afterwards (exact rewrite).

Expert MLP runs in fp16 single-pass (full PE rate, ~6x finer mantissa than
bf16), replacing the old 3+3-pass split-bf16 scheme (3x less PE work); fp16
weights are streamed in wide tiles (>=512B rows, full DMA rate).
AllReduce add-back is deferred to the next use of the block so the collective
overlaps the other block's compute instead of head-of-line-blocking the DVE
queue.  Final tied-embedding matmul in fp16.
"""

import os
import numpy as np

DBG_HOPS = int(os.environ.get("KDBG_HOPS", "4"))
DBG_FINAL = int(os.environ.get("KDBG_FINAL", "1"))
DBG_GATHER = int(os.environ.get("KDBG_GATHER", "1"))
DBG_AR = int(os.environ.get("KDBG_AR", "1"))
DBG_STAGE = int(os.environ.get("KDBG_STAGE", "99"))

V, D, E, H, T = 32000, 1024, 8, 4096, 2048
NHOPS = 4
NCORES = 8
VS = V // NCORES      # vocab shard per core
TB = 1024             # token block (collective granularity)
NB = T // TB
SB = 512              # MLP sub-block (psum N)
EPS = 1e-5

_CACHE = {}


def _build():
    import concourse.bass as bass
    import concourse.mybir as mybir
    import concourse.tile as tile
    from concourse import bacc
    from concourse.masks import make_identity

    dt = mybir.dt
    f32, i32 = dt.float32, dt.int32
    bf16 = dt.bfloat16
    fp16 = dt.float16
    AF = mybir.ActivationFunctionType
    Alu = mybir.AluOpType

    nc = bacc.Bacc(num_devices=NCORES)

    ids = nc.declare_dram_parameter("ids", [T], i32, isOutput=False)
    embed = nc.declare_dram_parameter("embed", [V, D], f32, isOutput=False)
    embT = nc.declare_dram_parameter("embT", [D, VS], fp16, isOutput=False)
    rw = nc.declare_dram_parameter("rw", [NHOPS, D, E], f32, isOutput=False)
    w1 = nc.declare_dram_parameter("w1", [D, H], fp16, isOutput=False)
    w2 = nc.declare_dram_parameter("w2", [H, D], fp16, isOutput=False)
    sel = nc.declare_dram_parameter("sel", [E], f32, isOutput=False)
    out = nc.declare_dram_parameter("out", [T, VS], f32, isOutput=True)

    KD = D // 128   # 8 d-chunks
    KH = H // 128   # 32 h-chunks
    NT = T // 128   # 16 token tiles
    NV = 8          # vocab chunks
    VC = VS // NV   # 500

    with tile.TileContext(nc) as tc:
        with tc.tile_pool(name="persist", bufs=1) as per:
            identity = per.tile([128, 128], f32)
            make_identity(nc, identity)

            ids_sb = per.tile([128, NT], i32)
            nc.sync.dma_start(out=ids_sb, in_=ids[:].rearrange("(n p) -> p n", p=128))

            sel_sb = per.tile([128, E], f32)
            sel_ap = sel[:]
            nc.sync.dma_start(
                out=sel_sb,
                in_=bass.AP(tensor=sel_ap.tensor, offset=0, ap=[[0, 128], [1, E]]),
            )

            eps1 = per.tile([1, 1], f32)
            nc.vector.memset(eps1, EPS)

            ones_k_f = per.tile([128, 1], f32)
            nc.vector.memset(ones_k_f, 1.0)
            ones_k = per.tile([128, 1], fp16)
            nc.vector.tensor_copy(out=ones_k[:], in_=ones_k_f[:])
            ones_1 = per.tile([1, 128], f32)
            nc.vector.memset(ones_1, 1.0)

            rw_sb = []
            for hop in range(NHOPS):
                t_ = per.tile([128, KD, E], f32, name=f"rw_sb{hop}")
                nc.sync.dma_start(
                    out=t_, in_=rw[hop].rearrange("(kd p) e -> p kd e", p=128)
                )
                rw_sb.append(t_)

            # persistent feature-major hidden state: h_T[d] = [128, T]
            h_T = [per.tile([128, T], f32, name=f"h_T{d}") for d in range(KD)]

            # ---- embedding gather + transpose to feature-major ----
            with (
                tc.tile_pool(name="init", bufs=2) as init,
                tc.tile_pool(name="init_ps", bufs=2, space="PSUM") as init_ps,
            ):
                # warm PE's vector clock on identity (avoids multi-wait matmuls)
                warm_ps = init_ps.tile([128, 128], f32, tag="tr", bufs=3)
                nc.tensor.transpose(
                    out=warm_ps[:], in_=identity[:], identity=identity[:]
                )
                for p in range(NT):
                    h0 = init.tile([128, D], f32, tag="h0", bufs=3)
                    if DBG_GATHER:
                        nc.gpsimd.indirect_dma_start(
                            out=h0[:],
                            out_offset=None,
                            in_=embed[:],
                            in_offset=bass.IndirectOffsetOnAxis(
                                ap=ids_sb[:, p : p + 1], axis=0
                            ),
                        )
                    else:
                        nc.sync.dma_start(
                            out=h0, in_=embed[p * 128 : (p + 1) * 128, :]
                        )
                    for d in range(KD):
                        tr_ps = init_ps.tile([128, 128], f32, tag="tr", bufs=3)
                        nc.tensor.transpose(
                            out=tr_ps[:],
                            in_=h0[:, d * 128 : (d + 1) * 128],
                            identity=identity[:],
                        )
                        nc.vector.tensor_copy(
                            out=h_T[d][:, p * 128 : (p + 1) * 128], in_=tr_ps[:]
                        )

            # ---- helper: rmsnorm scale for a token block -> s_bc [128, TB] f32
            def scale_block(work, psum, b):
                t0 = b * TB
                srow = work.tile([1, TB], f32, tag="srow", bufs=1)
                for ch in range(TB // 512):
                    c0 = t0 + ch * 512
                    ss_ps = psum.tile([1, 512], f32, tag="ssps", bufs=1)
                    for d in range(KD):
                        hsq = work.tile([128, 512], fp16, tag="hsq", bufs=1)
                        nc.vector.tensor_mul(
                            hsq[:], h_T[d][:, c0 : c0 + 512], h_T[d][:, c0 : c0 + 512]
                        )
                        nc.tensor.matmul(
                            out=ss_ps[:],
                            lhsT=ones_k[:],
                            rhs=hsq[:],
                            start=(d == 0),
                            stop=(d == KD - 1),
                        )
                    # sqrt(ss/D + eps)
                    nc.scalar.activation(
                        out=srow[:, ch * 512 : (ch + 1) * 512],
                        in_=ss_ps[:],
                        func=AF.Sqrt,
                        bias=eps1[:],
                        scale=1.0 / D,
                    )
                srec = srow
                nc.vector.reciprocal(out=srec[:], in_=srow[:])
                s_bc = work.tile([128, TB], f32, tag="sbc", bufs=1)
                for ch in range(TB // 512):
                    sp = psum.tile([128, 512], f32, tag="smallps", bufs=1)
                    nc.tensor.matmul(
                        out=sp[:],
                        lhsT=ones_1[:],
                        rhs=srec[:, ch * 512 : (ch + 1) * 512],
                        start=True,
                        stop=True,
                    )
                    nc.vector.tensor_copy(
                        out=s_bc[:, ch * 512 : (ch + 1) * 512], in_=sp[:]
                    )
                return s_bc

            # ---- hops ----
            with (
                tc.tile_pool(name="work", bufs=2) as work,
                tc.tile_pool(name="hops_ps", bufs=2, space="PSUM") as psum,
                tc.tile_pool(name="dram", bufs=2, space="DRAM") as dramp,
            ):
                w1_r = w1[:].rearrange("(kd p) h -> p kd h", p=128)
                w2_r = w2[:].rearrange("(kh p) dd -> p kh dd", p=128)

                # deferred AllReduce add-back: block b -> DRAM tile to add
                pend = {}

                def drain(b, work):
                    if b not in pend:
                        return
                    src_buf = pend.pop(b)
                    for d in range(KD):
                        for ch in range(TB // 512):
                            csl = slice(b * TB + ch * 512, b * TB + (ch + 1) * 512)
                            ar_t = work.tile([128, 512], f32, tag="ar", bufs=2)
                            nc.sync.dma_start(
                                out=ar_t,
                                in_=src_buf[
                                    d * 128 : (d + 1) * 128,
                                    ch * 512 : (ch + 1) * 512,
                                ],
                            )
                            nc.vector.tensor_add(
                                h_T[d][:, csl], h_T[d][:, csl], ar_t[:]
                            )

                for hop in range(DBG_HOPS):
                    for b in range(NB):
                        drain(b, work)
                        t0 = b * TB
                        s_bc = scale_block(work, psum, b)
                        if DBG_STAGE < 3:
                            continue
                        # router logits (fp32), scale applied post-matmul
                        lg_sb = work.tile([E, TB], f32, tag="lg", bufs=1)
                        for ch in range(TB // 512):
                            c0 = t0 + ch * 512
                            lg_ps = psum.tile([E, 512], f32, tag="smallps", bufs=1)
                            for d in range(KD):
                                nc.tensor.matmul(
                                    out=lg_ps[:],
                                    lhsT=rw_sb[hop][:, d, :],
                                    rhs=h_T[d][:, c0 : c0 + 512],
                                    start=(d == 0),
                                    stop=(d == KD - 1),
                                )
                            nc.vector.tensor_mul(
                                lg_sb[:, ch * 512 : (ch + 1) * 512],
                                lg_ps[:],
                                s_bc[:E, ch * 512 : (ch + 1) * 512],
                            )

                        if DBG_STAGE < 4:
                            continue
                        # top-2 masked softmax -> this core's prob column
                        prow = work.tile([1, TB], f32, tag="prow", bufs=1)
                        for p in range(TB // 128):
                            lgt_ps = psum.tile([128, E], f32, tag="smallps", bufs=1)
                            nc.tensor.transpose(
                                out=lgt_ps[:],
                                in_=lg_sb[:, p * 128 : (p + 1) * 128],
                                identity=identity[:E, :E],
                            )
                            lgt = work.tile([128, E], f32, tag="lgt", bufs=3)
                            nc.vector.tensor_copy(out=lgt[:], in_=lgt_ps[:])
                            m1n = work.tile([128, 1], f32, tag="m1n", bufs=3)
                            nc.vector.tensor_reduce(
                                out=m1n[:],
                                in_=lgt[:],
                                axis=mybir.AxisListType.X,
                                op=Alu.max,
                                negate=True,
                            )
                            d1 = work.tile([128, E], f32, tag="d1", bufs=3)
                            nc.vector.tensor_scalar_add(d1[:], lgt[:], m1n[:])
                            mask1 = work.tile([128, E], f32, tag="mask1", bufs=3)
                            nc.vector.tensor_scalar(
                                mask1[:], d1[:], 0.0, None, op0=Alu.is_equal
                            )
                            masked = work.tile([128, E], f32, tag="masked", bufs=3)
                            nc.vector.scalar_tensor_tensor(
                                out=masked[:],
                                in0=mask1[:],
                                scalar=-1e30,
                                in1=lgt[:],
                                op0=Alu.mult,
                                op1=Alu.add,
                            )
                            m2 = work.tile([128, 1], f32, tag="m2", bufs=3)
                            nc.vector.tensor_reduce(
                                out=m2[:],
                                in_=masked[:],
                                axis=mybir.AxisListType.X,
                                op=Alu.max,
                            )
                            top2 = work.tile([128, E], f32, tag="top2", bufs=3)
                            nc.vector.tensor_scalar(
                                top2[:], lgt[:], m2[:], None, op0=Alu.is_ge
                            )
                            e_t = work.tile([128, E], f32, tag="e_t", bufs=3)
                            se = work.tile([128, 1], f32, tag="se", bufs=3)
                            nc.scalar.activation(
                                out=e_t[:], in_=d1[:], func=AF.Exp, accum_out=se[:]
                            )
                            rse = work.tile([128, 1], f32, tag="rse", bufs=3)
                            nc.vector.reciprocal(out=rse[:], in_=se[:])
                            probs = work.tile([128, E], f32, tag="probs", bufs=3)
                            nc.vector.scalar_tensor_tensor(
                                out=probs[:],
                                in0=e_t[:],
                                scalar=rse[:],
                                in1=top2[:],
                                op0=Alu.mult,
                                op1=Alu.mult,
                            )
                            pscr = work.tile([128, E], f32, tag="pscr", bufs=3)
                            pcol = work.tile([128, 1], f32, tag="pcol", bufs=3)
                            nc.vector.tensor_mul(pscr[:], probs[:], sel_sb[:])
                            nc.vector.reduce_sum(
                                out=pcol[:], in_=pscr[:], axis=mybir.AxisListType.X
                            )
                            nc.sync.dma_start(
                                out=prow[0:1, p * 128 : (p + 1) * 128], in_=pcol[:]
                            )

                        if DBG_STAGE < 5:
                            continue
                        pb = work.tile([128, TB], f32, tag="pb", bufs=1)
                        for ch in range(TB // 512):
                            pp = psum.tile([128, 512], f32, tag="smallps", bufs=1)
                            nc.tensor.matmul(
                                out=pp[:],
                                lhsT=ones_1[:],
                                rhs=prow[:, ch * 512 : (ch + 1) * 512],
                                start=True,
                                stop=True,
                            )
                            nc.vector.tensor_copy(
                                out=pb[:, ch * 512 : (ch + 1) * 512], in_=pp[:]
                            )

                        if DBG_STAGE < 6:
                            continue
                        # expert MLP + weighted delta (fp32r single-pass)
                        din = dramp.tile([D, TB], f32, tag="din", bufs=2)
                        dout = dramp.tile(
                            [D, TB], f32, tag="dout", bufs=2, addr_space="Shared"
                        )
                        nsub = TB // SB
                        for sub in range(nsub):
                            ssl = slice(t0 + sub * SB, t0 + (sub + 1) * SB)
                            bsl = slice(sub * SB, (sub + 1) * SB)
                            xn = []
                            for d in range(KD):
                                xn_ = work.tile(
                                    [128, SB], fp16, tag=f"xn{d}", bufs=1,
                                    name=f"xn_{d}",
                                )
                                nc.vector.tensor_mul(
                                    xn_[:], h_T[d][:, ssl], s_bc[:, bsl]
                                )
                                xn.append(xn_)
                            us = []
                            for g in range(KH // 4):
                                w1t = work.tile(
                                    [128, KD, 512], fp16, tag="w1t", bufs=2
                                )
                                nc.sync.dma_start(
                                    out=w1t,
                                    in_=w1_r[:, :, g * 512 : (g + 1) * 512],
                                )
                                for mm in range(4):
                                    m = g * 4 + mm
                                    u_ps = psum.tile(
                                        [128, SB], f32, tag="ups", bufs=3
                                    )
                                    for kd in range(KD):
                                        nc.tensor.matmul(
                                            out=u_ps[:],
                                            lhsT=w1t[
                                                :, kd, mm * 128 : (mm + 1) * 128
                                            ],
                                            rhs=xn[kd][:],
                                            start=(kd == 0),
                                            stop=(kd == KD - 1),
                                        )
                                    u_ = work.tile(
                                        [128, SB], fp16, tag=f"u{m}", bufs=1,
                                        name=f"u_{m}",
                                    )
                                    nc.scalar.activation(
                                        out=u_[:], in_=u_ps[:],
                                        func=AF.Gelu_apprx_tanh,
                                    )
                                    us.append(u_)
                            for q in range(4):
                                w2t = work.tile(
                                    [128, KH, 256], fp16, tag="w2t", bufs=2
                                )
                                nc.sync.dma_start(
                                    out=w2t,
                                    in_=w2_r[:, :, q * 256 : (q + 1) * 256],
                                )
                                for dd in range(2):
                                    d = q * 2 + dd
                                    y_ps = psum.tile(
                                        [128, SB], f32, tag="yps", bufs=2
                                    )
                                    for kh in range(KH):
                                        nc.tensor.matmul(
                                            out=y_ps[:],
                                            lhsT=w2t[
                                                :, kh, dd * 128 : (dd + 1) * 128
                                            ],
                                            rhs=us[kh][:],
                                            start=(kh == 0),
                                            stop=(kh == KH - 1),
                                        )
                                    delta = work.tile(
                                        [128, SB], f32, tag="delta", bufs=2
                                    )
                                    nc.vector.tensor_mul(
                                        delta[:], y_ps[:], pb[:, bsl]
                                    )
                                    nc.sync.dma_start(
                                        out=din[d * 128 : (d + 1) * 128, bsl],
                                        in_=delta[:],
                                    )

                        if DBG_AR:
                            nc.gpsimd.collective_compute(
                                "AllReduce",
                                Alu.add,
                                replica_groups=[list(range(NCORES))],
                                ins=[din.opt()],
                                outs=[dout.opt()],
                            )
                            pend[b] = dout
                        else:
                            pend[b] = din

                # dramp closes with this block: drain all pending add-backs
                for b in range(NB):
                    drain(b, work)

            # ---- final: out = rmsnorm(h) @ embT (out-ln folded into embT) ----
            if not DBG_FINAL:
                with tc.tile_pool(name="fin0", bufs=2) as fin0:
                    for t in range(NT):
                        dbg = fin0.tile([128, VS], f32, tag="dbg", bufs=2)
                        nc.vector.memset(dbg, 0.0)
                        for d in range(KD):
                            nc.vector.tensor_copy(
                                out=dbg[:, d * 128 : (d + 1) * 128],
                                in_=h_T[d][:, t * 128 : (t + 1) * 128],
                            )
                        nc.sync.dma_start(
                            out=out[t * 128 : (t + 1) * 128, :], in_=dbg[:]
                        )
            else:
                with (
                    tc.tile_pool(name="fin", bufs=2) as fin,
                    tc.tile_pool(name="fin_ps", bufs=2, space="PSUM") as fpsum,
                ):
                    xnf = [
                        fin.tile(
                            [128, T], fp16, tag=f"xnf{d}", bufs=1, name=f"xnf_{d}"
                        )
                        for d in range(KD)
                    ]
                    for b in range(NB):
                        tsl = slice(b * TB, (b + 1) * TB)
                        s_bc = scale_block(fin, fpsum, b)
                        for d in range(KD):
                            nc.vector.tensor_mul(
                                xnf[d][:, tsl], h_T[d][:, tsl], s_bc[:]
                            )

                    embT_r = embT[:].rearrange("(kd p) v -> p kd v", p=128)
                    for v in range(NV):
                        et = fin.tile([128, KD, VC], fp16, tag="et", bufs=3)
                        nc.sync.dma_start(
                            out=et, in_=embT_r[:, :, v * VC : (v + 1) * VC]
                        )
                        for t in range(NT):
                            o_ps = fpsum.tile([128, VC], f32, tag="ops", bufs=4)
                            for kd in range(KD):
                                nc.tensor.matmul(
                                    out=o_ps[:],
                                    lhsT=xnf[kd][:, t * 128 : (t + 1) * 128],
                                    rhs=et[:, kd, :],
                                    start=(kd == 0),
                                    stop=(kd == KD - 1),
                                )
                            o_sb = fin.tile([128, VC], f32, tag="osb", bufs=4)
                            nc.scalar.copy(out=o_sb[:], in_=o_ps[:])
                            nc.sync.dma_start(
                                out=out[
                                    t * 128 : (t + 1) * 128, v * VC : (v + 1) * VC
                                ],
                                in_=o_sb[:],
                            )

    nc.finalize()
    return nc


def _prep_inputs(ids, embed_W, router_ln_w, router_W, expert_ln_w, expert_W1,
                 expert_W2, ln_out_w):
    """Host-side sharding + layout/constant-folding prep. Returns in_maps."""
    import ml_dtypes

    ids = np.ascontiguousarray(np.asarray(ids, dtype=np.int32))
    embed_W = np.ascontiguousarray(np.asarray(embed_W, dtype=np.float32))
    # fold router ln weight into router weights (rmsnorm scale applied on-device)
    rwf = np.ascontiguousarray(
        np.asarray(router_W, np.float32)
        * np.asarray(router_ln_w, np.float32)[:, :, None]
    )
    # fold out-ln into transposed embedding
    embT_full = np.ascontiguousarray(
        (embed_W * np.asarray(ln_out_w, np.float32)[None, :]).T
    )

    in_maps = []
    for c in range(NCORES):
        w1f = np.ascontiguousarray(
            (np.asarray(expert_W1[c], np.float32)
             * np.asarray(expert_ln_w[c], np.float32)[:, None]).astype(np.float16)
        )
        w2f = np.ascontiguousarray(
            np.asarray(expert_W2[c], np.float32).astype(np.float16)
        )
        embTc = np.ascontiguousarray(
            embT_full[:, c * VS : (c + 1) * VS].astype(np.float16)
        )
        selc = np.zeros((E,), np.float32)
        selc[c] = 1.0
        in_maps.append(
            {
                "ids": ids,
                "embed": embed_W,
                "embT": embTc,
                "rw": rwf,
                "w1": w1f,
                "w2": w2f,
                "sel": selc,
            }
        )
    return in_maps


def _get_nc():
    if "nc" not in _CACHE:
        _CACHE["nc"] = _build()
    return _CACHE["nc"]


def run(inputs, trace=False):
    """Run on 8 NeuronCores; returns (full_output, BassKernelResults)."""
    from concourse.bass_utils import run_bass_kernel_spmd

    k = int(inputs.get("k", 2))
    assert k == 2, f"kernel hardcodes top-2 routing, got k={k}"
    in_maps = _prep_inputs(
        inputs["ids"], inputs["embed_W"], inputs["router_ln_w"], inputs["router_W"],
        inputs["expert_ln_w"], inputs["expert_W1"], inputs["expert_W2"],
        inputs["ln_out_w"],
    )
    nc = _get_nc()
    res = run_bass_kernel_spmd(nc, in_maps, list(range(NCORES)), trace=trace)
    full = np.concatenate([res.results[c]["out"] for c in range(NCORES)], axis=1)
    return full, res


def kernel(**inputs) -> np.ndarray:
    full, _ = run(inputs, trace=False)
    return full
